# revision 4
# baseline (speedup 1.0000x reference)
"""Trainium2 Bass kernel for nn_KLFocalLossColBERT — int5 wire, count-adaptive.

Reference computation (B=128, LQ=32, LD=256, D=128, NWAY=16, GAMMA=5):
  q  = l2norm(query_reps, axis=2)                       # over D
  d  = l2norm(doc_reps * doc_masks[..., None], axis=2)  # over Ld (token axis)
  sim = einsum('bqd,nbld->nbql', q, d)
  scores[b, n] = sum_q max_l sim
  logp = log_softmax(scores, -1); p = exp(logp); t = labels[:, :NWAY]
  loss = mean(exp(t) * (t - logp) * p**GAMMA)

The graded metric is warm wall-clock of kernel(**inputs). The axon tunnel to
the 8 remote NeuronCores streams ~40 MB/s aggregate (concurrent transfers
CONGEST it — measured 20 MB/s at 8 back-to-back puts vs 40 at one — so puts
must be spaced, which the per-slab host prep provides for free), and there is
~160 ms of fixed dispatch+fetch RTT. Bytes-on-the-wire dominate. Strategy:
  - int5 wire format for docs: the reference normalizes docs PER-FEATURE over
    the token axis, so any per-feature quantization scale cancels exactly in
    the normalization. A global-scale 31-level quantizer (y = round(v/a),
    a=0.165, clipped to [-15,15], stored as x = y+16) needs no scale
    metadata. Three 5-bit values pack into one uint16 (x0|x1<<5|x2<<10):
    0.67 bytes/elem. Offline sim vs the exact reference: rel err 7.6e-5;
    hardware: 5.1e-5 (int4 sim'd at 1.5e-2 — too close to the 2e-2 gate;
    the fp8 predecessor measured 7.9e-4).
  - token compaction, count-adaptive: ~50% of doc tokens are masked; only
    unmasked tokens ship. Each (n, bl) pair is padded to the max unmasked
    count over the 8 cores sharing the SPMD program (mean ~139 vs the
    dataset-max 152), packed back-to-back per slab with compile-time
    offsets; the program is specialized to the per-pair length table
    (recompiled if the masks change — first-call compile is seconds).
    Short cores get PADW filler rows (y=0, sim contribution 0). Since a
    pair may have NO pad row on its longest core, the reference's
    masked-column zeros (which participate in its max) are restored
    exactly by clamping the per-(q,pair) token max at 0 — one DVE op.
  - host per slab: stable argsort of the mask -> unmasked-first token
    order; u16 top-half view gather -> three 64K-entry LUTs (interval-
    midpoint centering; pre-shifted by 0/5/10 bits) -> packed words ->
    pad fixup via u16 wraparound; each slab goes to an async device_put
    so host work streams under the wire. Per-core streams are contiguous
    slices (axis-0 sharding; no slicing copies inside device_put).
  - device (per core, 16x16 (n, b_local) pairs): DMA packed u16, DVE 5-bit
    field extract (and/shift), ACT upcast with bias -16 -> y in bf16,
    2x PE transpose -> PSUM [d, l]; ACT square+accum -> per-feature sumsq;
    rsqrt folded into the small qT operand; bf16 PE matmul sim + DVE
    reduce_max; max-with-0 clamp; scores via ones-select matmul ->
    out [NWAY, BSL]; softmax/KL/focal tail on host ([128,16], microseconds)
"""

import os
import sys

import numpy as np

for _p in ("/opt/trn_rl_repo", "/root/.axon_site/_ro/trn_rl_repo"):
    if os.path.isdir(_p) and _p not in sys.path:
        sys.path.insert(0, _p)

import jax
import ml_dtypes
import concourse.bacc as bacc_mod
import concourse.mybir as mybir
from concourse import bass2jax
from concourse.masks import make_identity
from concourse.tile import TileContext
from jax.experimental.shard_map import shard_map
from jax.sharding import Mesh, PartitionSpec

F32 = mybir.dt.float32
BF16 = mybir.dt.bfloat16
FP8 = mybir.dt.float8e4
U16 = mybir.dt.uint16
AF = mybir.ActivationFunctionType
ALU = mybir.AluOpType

B, LQ, LD, D, NWAY = 128, 32, 256, 128, 16
GAMMA = 5
NCORES = 8
BSL = B // NCORES    # 16 examples per core (B-sharding)
NPAIR = NWAY * BSL   # 256 (n, b_local) pairs per core
NG = NPAIR // 4      # 64 groups of 4 pairs packed per PSUM tile
NSLAB = 8            # doc pipeline slabs along NWAY
NL = NWAY // NSLAB   # 2 docs per slab tensor
NGR = 26             # base-27 groups per token (5 features each; 26*5>=130)
QSTEP = 0.21         # 25-level quantizer step for ~N(0,1) doc values
NBIAS = 8388608      # 2^23: makes the balanced-digit sum non-negative
MAGIC = 12582912.0   # 1.5*2^23: x+MAGIC-MAGIC rounds f32 to nearest int

E4M3 = mybir.dt.np(FP8)  # ml_dtypes.float8_e4m3

U8 = mybir.dt.uint8

_LUTS = None


def _get_luts():
    """bf16-code -> balanced digit y*27^k (i32), y=clip(round(v/a),-12,12)."""
    global _LUTS
    if _LUTS is None:
        with np.errstate(invalid="ignore", over="ignore"):
            codes = (np.arange(65536, dtype=np.uint32) << 16) | 0x8000
            v = codes.view(np.float32)
            d = np.clip(np.round(v / QSTEP), -12, 12).astype(np.int32)
            _LUTS = tuple(d * (27 ** k) for k in range(5))
    return _LUTS


def _build_nc(L):
    """L: [NWAY, BSL] int per-pair token counts (shared across cores)."""
    # per-slab flat stream: pair (nl, bl) at offset off[kk][nl, bl]
    offs, tots = [], []
    for kk in range(NSLAB):
        o = np.zeros((NL, BSL), np.int64)
        acc = 0
        for nl in range(NL):
            for bl in range(BSL):
                o[nl, bl] = acc
                acc += int(L[kk * NL + nl, bl])
        offs.append(o)
        tots.append(acc)

    nc = bacc_mod.Bacc()
    doc_aps = []
    for kk in range(NSLAB):
        t = nc.dram_tensor(f"docm{kk}", [1, 3, tots[kk], NGR], U8,
                           kind="ExternalInput")
        doc_aps.append(t[0])
    qt_d = nc.dram_tensor("qt", [D, BSL * LQ], FP8, kind="ExternalInput")
    out_d = nc.dram_tensor("out", [NWAY, BSL], F32, kind="ExternalOutput")
    qt_ap, out_ap = qt_d[:], out_d[:]

    with TileContext(nc) as tc:
        with (
            tc.tile_pool(name="consts", bufs=1) as consts,
            tc.tile_pool(name="apool", bufs=4) as apool,
            tc.tile_pool(name="xpool", bufs=4) as xpool,
            tc.tile_pool(name="bpool", bufs=4) as bpool,
            tc.tile_pool(name="rpool", bufs=8) as rpool,
            tc.tile_pool(name="scratch", bufs=2) as scratch,
            tc.tile_pool(name="small", bufs=4) as small,
            tc.tile_pool(name="qpool", bufs=8) as qpool,
            tc.tile_pool(name="ps_dt", bufs=2, space="PSUM") as ps_dt,
            tc.tile_pool(name="ps_sim", bufs=2, space="PSUM") as ps_sim,
            tc.tile_pool(name="ps_misc", bufs=1, space="PSUM") as ps_misc,
        ):
            identb = consts.tile([128, 128], BF16, tag="identb")
            make_identity(nc, identb)
            # esel column k selects partition block [32k, 32k+32) (sum over q)
            esel = consts.tile([128, 4], F32)
            nc.vector.memset(esel, 0.0)
            for k in range(4):
                nc.vector.memset(esel[32 * k:32 * k + 32, k:k + 1], 1.0)

            # q^T for this core's b-slice: [128 d, 512 (b q)] fp8 -> bf16
            qt8 = consts.tile([D, BSL * LQ], FP8, tag="qt8")
            nc.sync.dma_start(out=qt8, in_=qt_ap)
            qtb = consts.tile([D, BSL * LQ], BF16, tag="qtb")
            nc.scalar.activation(qtb, qt8, AF.Copy)

            stage = consts.tile([128, NG], F32)
            stage2 = consts.tile([128, NG], F32)

            def unpack(b0, b1, b2, rows, tag):
                """b0..b2: u8 byte-plane tiles [rows, NGR] of the balanced
                base-27 payload N = sum(y_k 27^k) + 2^23. Recovers the five
                digit planes y_k into a bf16 [rows, D] tile using only
                mult/add (magic-constant rounding for the /27 steps)."""
                c0 = xpool.tile([128, NGR], F32, tag=tag + "c0")
                nc.scalar.activation(c0[:rows], b0, AF.Copy)
                c1 = xpool.tile([128, NGR], F32, tag=tag + "c1")
                nc.scalar.activation(c1[:rows], b1, AF.Copy)
                # fold the -2^23 bias into the top byte: (b2-128)*65536
                c2 = xpool.tile([128, NGR], F32, tag=tag + "c2")
                nc.scalar.activation(c2[:rows], b2, AF.Copy, bias=-128.0)
                X = xpool.tile([128, NGR], F32, tag=tag + "X0")
                nc.vector.scalar_tensor_tensor(
                    out=X[:rows], in0=c1[:rows], scalar=256.0, in1=c0[:rows],
                    op0=ALU.mult, op1=ALU.add)
                nc.vector.scalar_tensor_tensor(
                    out=X[:rows], in0=c2[:rows], scalar=65536.0, in1=X[:rows],
                    op0=ALU.mult, op1=ALU.add)
                Ab = bpool.tile([128, D], BF16, tag=tag + "ab")
                for k in range(4):
                    Xn = xpool.tile([128, NGR], F32, tag=f"{tag}X{k + 1}")
                    nc.vector.tensor_scalar(
                        out=Xn[:rows], in0=X[:rows], scalar1=1.0 / 27.0,
                        scalar2=MAGIC, op0=ALU.mult, op1=ALU.add)
                    nc.vector.tensor_scalar(
                        out=Xn[:rows], in0=Xn[:rows], scalar1=MAGIC,
                        scalar2=None, op0=ALU.subtract)
                    # digit y_k = X - 27*Xn, straight into its feature slice
                    nc.vector.scalar_tensor_tensor(
                        out=Ab[:rows, k * NGR:(k + 1) * NGR], in0=Xn[:rows],
                        scalar=-27.0, in1=X[:rows], op0=ALU.mult, op1=ALU.add)
                    X = Xn
                # y4 = X4 itself (top digit); only 24 groups carry features
                nc.scalar.activation(Ab[:rows, 4 * NGR:D], X[:rows, 0:D - 4 * NGR],
                                     AF.Copy)
                return Ab

            for g in range(NG):
                ssq = small.tile([128, 4], F32, tag="ssq")
                rtiles = []
                for k in range(4):
                    j = 4 * g + k
                    n, bl = j // BSL, j % BSL
                    kk, nl = n // NL, n % NL
                    ln = int(L[n, bl])
                    off = int(offs[kk][nl, bl])
                    r0 = min(128, ln)
                    r1 = ln - r0
                    dap = doc_aps[kk]
                    P00 = apool.tile([128, NGR], U8, tag="P00")
                    nc.sync.dma_start(out=P00[:r0], in_=dap[0, off:off + r0])
                    P01 = apool.tile([128, NGR], U8, tag="P01")
                    nc.sync.dma_start(out=P01[:r0], in_=dap[1, off:off + r0])
                    P02 = apool.tile([128, NGR], U8, tag="P02")
                    nc.sync.dma_start(out=P02[:r0], in_=dap[2, off:off + r0])
                    Ab0 = unpack(P00[:r0], P01[:r0], P02[:r0], r0, "u0")
                    pdt = ps_dt.tile([D, 152], BF16, tag="pdt")
                    nc.tensor.transpose(pdt[:, 0:r0], Ab0[:r0],
                                        identb[:r0, :r0])
                    if r1 > 0:
                        P10 = apool.tile([128, NGR], U8, tag="P10")
                        nc.sync.dma_start(out=P10[:r1],
                                          in_=dap[0, off + r0:off + ln])
                        P11 = apool.tile([128, NGR], U8, tag="P11")
                        nc.sync.dma_start(out=P11[:r1],
                                          in_=dap[1, off + r0:off + ln])
                        P12 = apool.tile([128, NGR], U8, tag="P12")
                        nc.sync.dma_start(out=P12[:r1],
                                          in_=dap[2, off + r0:off + ln])
                        Ab1 = unpack(P10[:r1], P11[:r1], P12[:r1], r1, "u1")
                        nc.tensor.transpose(pdt[:, r0:ln], Ab1[:r1],
                                            identb[:r1, :r1])
                    # per-feature sumsq over l (ACT square + free-axis accum)
                    sq = scratch.tile([D, 152], F32, tag="sq")
                    nc.scalar.activation(sq[:, :ln], pdt[:, :ln], AF.Square,
                                         accum_out=ssq[:, k:k + 1])
                    R = rpool.tile([D, 152], BF16, tag="R")
                    nc.vector.tensor_copy(R[:, :ln], pdt[:, :ln])
                    rtiles.append((R, ln))

                nrm = small.tile([128, 4], F32, tag="nrm")
                nc.scalar.activation(nrm, ssq, AF.Sqrt)
                rinv = small.tile([128, 4], F32, tag="rinv")
                nc.vector.reciprocal(rinv, nrm)

                psim = ps_sim.tile([128, 152], F32, tag="psim")
                for k in range(4):
                    bl = (4 * g + k) % BSL
                    R, ln = rtiles[k]
                    qTs = qpool.tile([D, LQ], BF16, tag="qTs")
                    nc.vector.tensor_scalar_mul(
                        qTs, qtb[:, bl * LQ:(bl + 1) * LQ], rinv[:, k:k + 1]
                    )
                    nc.tensor.matmul(
                        psim[32 * k:32 * k + 32, :ln], lhsT=qTs,
                        rhs=R[:, :ln],
                        start=True, stop=True, tile_position=(0, 32 * k),
                    )
                    nc.vector.reduce_max(
                        stage[32 * k:32 * k + 32, g:g + 1],
                        psim[32 * k:32 * k + 32, :ln],
                        axis=mybir.AxisListType.X,
                    )

            # reference max runs over masked columns (sim 0) too: clamp at 0
            nc.vector.tensor_scalar(out=stage2, in0=stage, scalar1=0.0,
                                    scalar2=None, op0=ALU.max)

            # scores: esel^T @ stage2 -> [4, NG]; sc[k, g] = score of pair
            # j=4g+k, i.e. out[n=g//4, bl=4*(g%4)+k]; one scatter DMA
            ps_sc = ps_misc.tile([4, NG], F32, tag="misc")
            nc.tensor.matmul(ps_sc, lhsT=esel, rhs=stage2, start=True,
                             stop=True)
            sc_row = small.tile([4, NG], F32, tag="scrow")
            nc.vector.tensor_copy(sc_row, ps_sc)
            nc.sync.dma_start(
                out=out_ap.rearrange("n (g2 k) -> k (n g2)", k=4),
                in_=sc_row,
            )

    nc.finalize()
    return nc


_CACHE: dict = {}


def _get_runner(L):
    key = L.tobytes()
    if _CACHE.get("key") == key:
        return _CACHE["fn"]

    bass2jax.install_neuronx_cc_hook()
    nc = _build_nc(L)

    partition_name = (
        nc.partition_id_tensor.name if nc.partition_id_tensor else None
    )
    in_names: list[str] = []
    out_names: list[str] = []
    out_avals: list[jax.core.ShapedArray] = []
    for alloc in nc.m.functions[0].allocations:
        if not isinstance(alloc, mybir.MemoryLocationSet):
            continue
        name = alloc.memorylocations[0].name
        if alloc.kind == "ExternalInput":
            if name != partition_name:
                in_names.append(name)
        elif alloc.kind == "ExternalOutput":
            out_names.append(name)
            shape = tuple(alloc.tensor_shape)
            dtype = mybir.dt.np(alloc.dtype)
            out_avals.append(jax.core.ShapedArray(shape, dtype))
    name_order = list(in_names) + list(out_names)
    in_names = in_names + out_names
    if partition_name is not None:
        in_names.append(partition_name)

    def _body(*args):
        operands = list(args)
        if partition_name is not None:
            operands.append(bass2jax.partition_id_tensor())
        outs = bass2jax._bass_exec_p.bind(
            *operands,
            out_avals=tuple(out_avals),
            in_names=tuple(in_names),
            out_names=tuple(out_names),
            lowering_input_output_aliases=(),
            sim_require_finite=True,
            sim_require_nnan=True,
            nc=nc,
        )
        return tuple(outs)

    if "mesh" not in _CACHE:
        devices = jax.devices()[:NCORES]
        _CACHE["mesh"] = Mesh(np.asarray(devices), ("core",))
    mesh = _CACHE["mesh"]
    # doc streams shard along the core axis (axis 0); qt/out along axis 1
    spec_doc = PartitionSpec("core", None, None, None)
    spec_col = PartitionSpec(None, "core")
    specs_by_name = {}
    for nm in name_order:
        specs_by_name[nm] = spec_doc if nm.startswith("docm") else spec_col
    in_specs = tuple(specs_by_name[nm] for nm in name_order)
    sharded = jax.jit(
        shard_map(_body, mesh=mesh, in_specs=in_specs, out_specs=(spec_col,),
                  check_rep=False),
        keep_unused=True,
    )
    from jax.sharding import NamedSharding

    _CACHE["key"] = key
    _CACHE["fn"] = sharded
    _CACHE["shard_doc"] = NamedSharding(mesh, spec_doc)
    _CACHE["shard_col"] = NamedSharding(mesh, spec_col)
    return sharded


def _cast_slab(d2_u16, flat_sel, m_sel):
    """flat_sel [8, T] global token rows; m_sel [8, T] int32 validity.
    Returns byte planes [8, 3, T, NGR] u8 of N = sum(y_k 27^k) + 2^23
    (pads -> all digits 0, i.e. N = 2^23)."""
    l0, l1, l2, l3, l4 = _get_luts()
    g16 = d2_u16[flat_sel]                    # [8, T, 128] u16
    n = l0[g16[..., 0:NGR]]
    n += l1[g16[..., NGR:2 * NGR]]
    n += l2[g16[..., 2 * NGR:3 * NGR]]
    n += l3[g16[..., 3 * NGR:4 * NGR]]
    n[..., 0:D - 4 * NGR] += l4[g16[..., 4 * NGR:D]]
    n *= m_sel[..., None]                     # pads -> 0 (all digits 0)
    n += NBIAS
    # little-endian byte planes: strided u8 view copies, no arithmetic
    u8v = n.view(np.uint8).reshape(n.shape + (4,))
    out = np.empty((n.shape[0], 3) + n.shape[1:], np.uint8)
    out[:, 0] = u8v[..., 0]
    out[:, 1] = u8v[..., 1]
    out[:, 2] = u8v[..., 2]
    return out


def _prep_q(q: np.ndarray) -> np.ndarray:
    """L2-normalize over D, transpose to [D, B*LQ], cast to e4m3."""
    nrm = np.sqrt((q.astype(np.float64) ** 2).sum(-1, keepdims=True))
    qn = (q / np.maximum(nrm, 1e-12)).astype(np.float32)
    qt = np.ascontiguousarray(qn.transpose(2, 0, 1).reshape(D, B * LQ))
    return qt.astype(E4M3)


def _tail(scores: np.ndarray, lab: np.ndarray) -> np.float32:
    """softmax / KL / focal on [B, NWAY] in float64."""
    sc = scores.astype(np.float64)
    m = sc.max(-1, keepdims=True)
    ls = np.log(np.exp(sc - m).sum(-1, keepdims=True)) + m
    logp = sc - ls
    p = np.exp(logp)
    t = lab[:, :NWAY].astype(np.float64)
    kl = np.exp(t) * (t - logp)
    lv = kl * p ** GAMMA
    return np.float32(lv.mean())


def run(inputs, trace=False):
    q = np.asarray(inputs["query_reps"], dtype=np.float32)
    doc = np.ascontiguousarray(
        np.asarray(inputs["doc_reps"], dtype=np.float32)
    )
    msk = np.asarray(inputs["doc_masks"])
    lab = np.asarray(inputs["labels"], dtype=np.float32)

    if "base" not in _CACHE:
        _CACHE["base"] = (
            (np.arange(NWAY, dtype=np.int32)[:, None, None] * B
             + np.arange(B, dtype=np.int32)[None, :, None]) * LD
        )
    for attempt in range(3):
        loss = _attempt(q, doc, msk, lab)
        # the axon tunnel rarely corrupts a transfer (observed transient
        # NaN); the loss is one scalar, so detect and retry
        if np.isfinite(loss):
            break

    class _Res:
        results = None
        instructions_and_trace = None
        profile_json = None
        exec_time_ns = None

    return np.array(loss, dtype=np.float32), _Res()


def _attempt(q, doc, msk, lab):
    base = _CACHE["base"]
    # per-pair shared lengths: max unmasked count over the 8 cores
    cnt = msk.sum(-1, dtype=np.int32)                  # [NWAY, B]
    L = cnt.reshape(NWAY, NCORES, BSL).max(1)          # [NWAY, BSL]
    fn = _get_runner(L)
    if "zeros_dev" not in _CACHE:
        _CACHE["zeros_dev"] = jax.device_put(
            np.zeros((NWAY, B), np.float32), _CACHE["shard_col"])
    # top 16 bits of each f32 as strided rows [NWAY*B*LD, D]
    d2_u16 = doc.view(np.uint16)[..., 1::2].reshape(-1, D)
    qt_dev = jax.device_put(_prep_q(q), _CACHE["shard_col"])
    slabs = []
    ar = np.arange(LD, dtype=np.int32)
    for kk in range(NSLAB):
        sl = slice(kk * NL, (kk + 1) * NL)  # docs {2kk, 2kk+1}, all b
        Ls = L[sl]                                      # [NL, BSL]
        lmax = int(Ls.max())
        # regroup the mask to core-major [8, NL, BSL, LD] FIRST (small,
        # contiguous copy) so argsort/take outputs need no big transposes
        msk_t = np.ascontiguousarray(
            msk[sl].reshape(NL, NCORES, BSL, LD).transpose(1, 0, 2, 3))
        # stable sort puts ALL unmasked tokens first, in original order
        o = np.argsort(1 - msk_t, axis=-1, kind="stable")[..., :lmax]
        o = o.astype(np.int32)
        m16 = np.take_along_axis(msk_t, o, axis=-1).astype(np.int32)
        # global token-row base per (core, nl, bl)
        bt = ((kk * NL + np.arange(NL, dtype=np.int32))[None, :, None] * B
              + (np.arange(NCORES, dtype=np.int32)[:, None, None] * BSL
                 + np.arange(BSL, dtype=np.int32)[None, None, :])) * LD
        flat = bt[..., None] + o                        # [8, NL, BSL, lmax]
        # ragged-select the first L[nl, bl] positions of each pair
        sel = ar[None, None, :lmax] < Ls[:, :, None]    # [NL, BSL, lmax]
        flat_sel = flat[:, sel]                         # [8, T]
        m_sel = m16[:, sel]
        p16 = _cast_slab(d2_u16, flat_sel, m_sel)       # [8, 3, T, NGR]
        slabs.append(jax.device_put(p16, _CACHE["shard_doc"]))
    (outg,) = fn(*slabs, qt_dev, _CACHE["zeros_dev"])
    scores = np.asarray(outg).astype(np.float32).T  # [B, NWAY]
    return _tail(scores, lab)


def kernel(**inputs) -> np.ndarray:
    out, _ = run(inputs)
    return out


# revision 7
# speedup vs baseline: 1.0928x; 1.0928x over previous
"""Trainium2 Bass kernel for nn_KLFocalLossColBERT — int5 wire, count-adaptive.

Reference computation (B=128, LQ=32, LD=256, D=128, NWAY=16, GAMMA=5):
  q  = l2norm(query_reps, axis=2)                       # over D
  d  = l2norm(doc_reps * doc_masks[..., None], axis=2)  # over Ld (token axis)
  sim = einsum('bqd,nbld->nbql', q, d)
  scores[b, n] = sum_q max_l sim
  logp = log_softmax(scores, -1); p = exp(logp); t = labels[:, :NWAY]
  loss = mean(exp(t) * (t - logp) * p**GAMMA)

The graded metric is warm wall-clock of kernel(**inputs). The axon tunnel to
the 8 remote NeuronCores streams ~40 MB/s aggregate (concurrent transfers
CONGEST it — measured 20 MB/s at 8 back-to-back puts vs 40 at one — so puts
must be spaced, which the per-slab host prep provides for free), and there is
~160 ms of fixed dispatch+fetch RTT. Bytes-on-the-wire dominate. Strategy:
  - balanced base-27 wire format for docs: the reference normalizes docs
    PER-FEATURE over the token axis, so any per-feature quantization scale
    cancels exactly in the normalization. A global-scale 25-level quantizer
    (y = round(v/a), a=0.21, clipped to [-12,12]) needs no scale metadata.
    FIVE balanced digits pack into one 24-bit payload N = sum(y_k 27^k)+2^23,
    shipped as three u8 byte planes: 0.6 bytes/elem. The device recovers the
    digits with only mult/add/sub (the compiler rejects mod/divide):
    X+1.5*2^23-1.5*2^23 rounds f32 to the nearest integer, |y|<=12 keeps
    round(X/27) exact, y_k = X_k - 27*X_{k+1}. End-to-end rel err 7.1e-3
    (hardware == offline numpy sim bit-exactly; the 2e-2 gate has 2.8x
    margin; int4 sim'd at 1.5e-2 — too close; fp8 measured 7.9e-4).
  - token compaction, count-adaptive: ~50% of doc tokens are masked; only
    unmasked tokens ship. Each (n, bl) pair is padded to the max unmasked
    count over the 8 cores sharing the SPMD program (mean ~139 vs the
    dataset-max 152), packed back-to-back per slab with compile-time
    offsets; the program is specialized to the per-pair length table
    (recompiled if the masks change — first-call compile is seconds).
    Short cores get PADW filler rows (y=0, sim contribution 0). Since a
    pair may have NO pad row on its longest core, the reference's
    masked-column zeros (which participate in its max) are restored
    exactly by clamping the per-(q,pair) token max at 0 — one DVE op.
  - host per slab: stable argsort of the mask -> unmasked-first token
    order; u16 top-half view gather -> three 64K-entry LUTs (interval-
    midpoint centering; pre-shifted by 0/5/10 bits) -> packed words ->
    pad fixup via u16 wraparound; each slab goes to an async device_put
    so host work streams under the wire. Per-core streams are contiguous
    slices (axis-0 sharding; no slicing copies inside device_put).
  - device (per core, 16x16 (n, b_local) pairs): DMA packed u16, DVE 5-bit
    field extract (and/shift), ACT upcast with bias -16 -> y in bf16,
    2x PE transpose -> PSUM [d, l]; ACT square+accum -> per-feature sumsq;
    rsqrt folded into the small qT operand; bf16 PE matmul sim + DVE
    reduce_max; max-with-0 clamp; scores via ones-select matmul ->
    out [NWAY, BSL]; softmax/KL/focal tail on host ([128,16], microseconds)
"""

import os
import sys

import numpy as np

for _p in ("/opt/trn_rl_repo", "/root/.axon_site/_ro/trn_rl_repo"):
    if os.path.isdir(_p) and _p not in sys.path:
        sys.path.insert(0, _p)

import jax
import ml_dtypes
import concourse.bacc as bacc_mod
import concourse.mybir as mybir
from concourse import bass2jax
from concourse.masks import make_identity
from concourse.tile import TileContext
from jax.experimental.shard_map import shard_map
from jax.sharding import Mesh, PartitionSpec

F32 = mybir.dt.float32
BF16 = mybir.dt.bfloat16
FP8 = mybir.dt.float8e4
U16 = mybir.dt.uint16
AF = mybir.ActivationFunctionType
ALU = mybir.AluOpType

B, LQ, LD, D, NWAY = 128, 32, 256, 128, 16
GAMMA = 5
NCORES = 8
BSL = B // NCORES    # 16 examples per core (B-sharding)
NPAIR = NWAY * BSL   # 256 (n, b_local) pairs per core
NG = NPAIR // 4      # 64 groups of 4 pairs packed per PSUM tile
NSLAB = 8            # doc pipeline slabs along NWAY
NL = NWAY // NSLAB   # 2 docs per slab tensor
NGR = 26             # base-27 groups per token (5 features each; 26*5>=130)
QSTEP = 0.21         # 25-level quantizer step for ~N(0,1) doc values
NBIAS = 8388608      # 2^23: makes the balanced-digit sum non-negative
MAGIC = 12582912.0   # 1.5*2^23: x+MAGIC-MAGIC rounds f32 to nearest int

E4M3 = mybir.dt.np(FP8)  # ml_dtypes.float8_e4m3

U8 = mybir.dt.uint8

_LUTS = None


def _get_luts():
    """bf16-code -> balanced digit y*27^k (i32), y=clip(round(v/a),-12,12)."""
    global _LUTS
    if _LUTS is None:
        with np.errstate(invalid="ignore", over="ignore"):
            codes = (np.arange(65536, dtype=np.uint32) << 16) | 0x8000
            v = codes.view(np.float32)
            d = np.clip(np.round(v / QSTEP), -12, 12).astype(np.int32)
            _LUTS = tuple(d * (27 ** k) for k in range(5))
    return _LUTS


def _build_nc(L):
    """L: [NWAY, BSL] int per-pair token counts (shared across cores)."""
    # per-slab flat stream: pair (nl, bl) at offset off[kk][nl, bl]
    offs, tots = [], []
    for kk in range(NSLAB):
        o = np.zeros((NL, BSL), np.int64)
        acc = 0
        for nl in range(NL):
            for bl in range(BSL):
                o[nl, bl] = acc
                acc += int(L[kk * NL + nl, bl])
        offs.append(o)
        tots.append(acc)

    nc = bacc_mod.Bacc()
    doc_aps = []
    for kk in range(NSLAB):
        t = nc.dram_tensor(f"docm{kk}", [1, 3, tots[kk], NGR], U8,
                           kind="ExternalInput")
        doc_aps.append(t[0])
    qt_d = nc.dram_tensor("qt", [D, BSL * LQ], FP8, kind="ExternalInput")
    out_d = nc.dram_tensor("out", [NWAY, BSL], F32, kind="ExternalOutput")
    qt_ap, out_ap = qt_d[:], out_d[:]

    with TileContext(nc) as tc:
        with (
            tc.tile_pool(name="consts", bufs=1) as consts,
            tc.tile_pool(name="apool", bufs=4) as apool,
            tc.tile_pool(name="xpool", bufs=4) as xpool,
            tc.tile_pool(name="bpool", bufs=4) as bpool,
            tc.tile_pool(name="rpool", bufs=8) as rpool,
            tc.tile_pool(name="scratch", bufs=2) as scratch,
            tc.tile_pool(name="small", bufs=4) as small,
            tc.tile_pool(name="qpool", bufs=8) as qpool,
            tc.tile_pool(name="ps_dt", bufs=2, space="PSUM") as ps_dt,
            tc.tile_pool(name="ps_sim", bufs=2, space="PSUM") as ps_sim,
            tc.tile_pool(name="ps_misc", bufs=1, space="PSUM") as ps_misc,
        ):
            identb = consts.tile([128, 128], BF16, tag="identb")
            make_identity(nc, identb)
            # esel column k selects partition block [32k, 32k+32) (sum over q)
            esel = consts.tile([128, 4], F32)
            nc.vector.memset(esel, 0.0)
            for k in range(4):
                nc.vector.memset(esel[32 * k:32 * k + 32, k:k + 1], 1.0)

            # q^T for this core's b-slice: [128 d, 512 (b q)] fp8 -> bf16
            qt8 = consts.tile([D, BSL * LQ], FP8, tag="qt8")
            nc.sync.dma_start(out=qt8, in_=qt_ap)
            qtb = consts.tile([D, BSL * LQ], BF16, tag="qtb")
            nc.scalar.activation(qtb, qt8, AF.Copy)

            stage = consts.tile([128, NG], F32)
            stage2 = consts.tile([128, NG], F32)

            def unpack(b0, b1, b2, rows, tag):
                """b0..b2: u8 byte-plane tiles [rows, NGR] of the balanced
                base-27 payload N = sum(y_k 27^k) + 2^23. Recovers the five
                digit planes y_k into a bf16 [rows, D] tile using only
                mult/add (magic-constant rounding for the /27 steps)."""
                c0 = xpool.tile([128, NGR], F32, tag=tag + "c0")
                nc.scalar.activation(c0[:rows], b0, AF.Copy)
                c1 = xpool.tile([128, NGR], F32, tag=tag + "c1")
                nc.scalar.activation(c1[:rows], b1, AF.Copy)
                # fold the -2^23 bias into the top byte: (b2-128)*65536
                c2 = xpool.tile([128, NGR], F32, tag=tag + "c2")
                nc.scalar.activation(c2[:rows], b2, AF.Copy, bias=-128.0)
                X = xpool.tile([128, NGR], F32, tag=tag + "X0")
                nc.vector.scalar_tensor_tensor(
                    out=X[:rows], in0=c1[:rows], scalar=256.0, in1=c0[:rows],
                    op0=ALU.mult, op1=ALU.add)
                nc.vector.scalar_tensor_tensor(
                    out=X[:rows], in0=c2[:rows], scalar=65536.0, in1=X[:rows],
                    op0=ALU.mult, op1=ALU.add)
                Ab = bpool.tile([128, D], BF16, tag=tag + "ab")
                for k in range(4):
                    Xn = xpool.tile([128, NGR], F32, tag=f"{tag}X{k + 1}")
                    nc.vector.tensor_scalar(
                        out=Xn[:rows], in0=X[:rows], scalar1=1.0 / 27.0,
                        scalar2=MAGIC, op0=ALU.mult, op1=ALU.add)
                    nc.vector.tensor_scalar(
                        out=Xn[:rows], in0=Xn[:rows], scalar1=MAGIC,
                        scalar2=None, op0=ALU.subtract)
                    # digit y_k = X - 27*Xn, straight into its feature slice
                    nc.vector.scalar_tensor_tensor(
                        out=Ab[:rows, k * NGR:(k + 1) * NGR], in0=Xn[:rows],
                        scalar=-27.0, in1=X[:rows], op0=ALU.mult, op1=ALU.add)
                    X = Xn
                # y4 = X4 itself (top digit); only 24 groups carry features
                nc.scalar.activation(Ab[:rows, 4 * NGR:D], X[:rows, 0:D - 4 * NGR],
                                     AF.Copy)
                return Ab

            for g in range(NG):
                ssq = small.tile([128, 4], F32, tag="ssq")
                rtiles = []
                for k in range(4):
                    j = 4 * g + k
                    n, bl = j // BSL, j % BSL
                    kk, nl = n // NL, n % NL
                    ln = int(L[n, bl])
                    off = int(offs[kk][nl, bl])
                    r0 = min(128, ln)
                    r1 = ln - r0
                    dap = doc_aps[kk]
                    P00 = apool.tile([128, NGR], U8, tag="P00")
                    nc.sync.dma_start(out=P00[:r0], in_=dap[0, off:off + r0])
                    P01 = apool.tile([128, NGR], U8, tag="P01")
                    nc.sync.dma_start(out=P01[:r0], in_=dap[1, off:off + r0])
                    P02 = apool.tile([128, NGR], U8, tag="P02")
                    nc.sync.dma_start(out=P02[:r0], in_=dap[2, off:off + r0])
                    Ab0 = unpack(P00[:r0], P01[:r0], P02[:r0], r0, "u0")
                    pdt = ps_dt.tile([D, 152], BF16, tag="pdt")
                    nc.tensor.transpose(pdt[:, 0:r0], Ab0[:r0],
                                        identb[:r0, :r0])
                    if r1 > 0:
                        P10 = apool.tile([128, NGR], U8, tag="P10")
                        nc.sync.dma_start(out=P10[:r1],
                                          in_=dap[0, off + r0:off + ln])
                        P11 = apool.tile([128, NGR], U8, tag="P11")
                        nc.sync.dma_start(out=P11[:r1],
                                          in_=dap[1, off + r0:off + ln])
                        P12 = apool.tile([128, NGR], U8, tag="P12")
                        nc.sync.dma_start(out=P12[:r1],
                                          in_=dap[2, off + r0:off + ln])
                        Ab1 = unpack(P10[:r1], P11[:r1], P12[:r1], r1, "u1")
                        nc.tensor.transpose(pdt[:, r0:ln], Ab1[:r1],
                                            identb[:r1, :r1])
                    # per-feature sumsq over l (ACT square + free-axis accum)
                    sq = scratch.tile([D, 152], F32, tag="sq")
                    nc.scalar.activation(sq[:, :ln], pdt[:, :ln], AF.Square,
                                         accum_out=ssq[:, k:k + 1])
                    R = rpool.tile([D, 152], BF16, tag="R")
                    nc.vector.tensor_copy(R[:, :ln], pdt[:, :ln])
                    rtiles.append((R, ln))

                nrm = small.tile([128, 4], F32, tag="nrm")
                nc.scalar.activation(nrm, ssq, AF.Sqrt)
                rinv = small.tile([128, 4], F32, tag="rinv")
                nc.vector.reciprocal(rinv, nrm)

                psim = ps_sim.tile([128, 152], F32, tag="psim")
                for k in range(4):
                    bl = (4 * g + k) % BSL
                    R, ln = rtiles[k]
                    qTs = qpool.tile([D, LQ], BF16, tag="qTs")
                    nc.vector.tensor_scalar_mul(
                        qTs, qtb[:, bl * LQ:(bl + 1) * LQ], rinv[:, k:k + 1]
                    )
                    nc.tensor.matmul(
                        psim[32 * k:32 * k + 32, :ln], lhsT=qTs,
                        rhs=R[:, :ln],
                        start=True, stop=True, tile_position=(0, 32 * k),
                    )
                    nc.vector.reduce_max(
                        stage[32 * k:32 * k + 32, g:g + 1],
                        psim[32 * k:32 * k + 32, :ln],
                        axis=mybir.AxisListType.X,
                    )

            # reference max runs over masked columns (sim 0) too: clamp at 0
            nc.vector.tensor_scalar(out=stage2, in0=stage, scalar1=0.0,
                                    scalar2=None, op0=ALU.max)

            # scores: esel^T @ stage2 -> [4, NG]; sc[k, g] = score of pair
            # j=4g+k, i.e. out[n=g//4, bl=4*(g%4)+k]; one scatter DMA
            ps_sc = ps_misc.tile([4, NG], F32, tag="misc")
            nc.tensor.matmul(ps_sc, lhsT=esel, rhs=stage2, start=True,
                             stop=True)
            sc_row = small.tile([4, NG], F32, tag="scrow")
            nc.vector.tensor_copy(sc_row, ps_sc)
            nc.sync.dma_start(
                out=out_ap.rearrange("n (g2 k) -> k (n g2)", k=4),
                in_=sc_row,
            )

    nc.finalize()
    return nc


_CACHE: dict = {}


def _get_runner(L):
    key = L.tobytes()
    if _CACHE.get("key") == key:
        return _CACHE["fn"]

    bass2jax.install_neuronx_cc_hook()
    nc = _build_nc(L)

    partition_name = (
        nc.partition_id_tensor.name if nc.partition_id_tensor else None
    )
    in_names: list[str] = []
    out_names: list[str] = []
    out_avals: list[jax.core.ShapedArray] = []
    for alloc in nc.m.functions[0].allocations:
        if not isinstance(alloc, mybir.MemoryLocationSet):
            continue
        name = alloc.memorylocations[0].name
        if alloc.kind == "ExternalInput":
            if name != partition_name:
                in_names.append(name)
        elif alloc.kind == "ExternalOutput":
            out_names.append(name)
            shape = tuple(alloc.tensor_shape)
            dtype = mybir.dt.np(alloc.dtype)
            out_avals.append(jax.core.ShapedArray(shape, dtype))
    name_order = list(in_names) + list(out_names)
    in_names = in_names + out_names
    if partition_name is not None:
        in_names.append(partition_name)

    def _body(*args):
        operands = list(args)
        if partition_name is not None:
            operands.append(bass2jax.partition_id_tensor())
        outs = bass2jax._bass_exec_p.bind(
            *operands,
            out_avals=tuple(out_avals),
            in_names=tuple(in_names),
            out_names=tuple(out_names),
            lowering_input_output_aliases=(),
            sim_require_finite=True,
            sim_require_nnan=True,
            nc=nc,
        )
        return tuple(outs)

    if "mesh" not in _CACHE:
        devices = jax.devices()[:NCORES]
        _CACHE["mesh"] = Mesh(np.asarray(devices), ("core",))
    mesh = _CACHE["mesh"]
    # doc streams shard along the core axis (axis 0); qt/out along axis 1
    spec_doc = PartitionSpec("core", None, None, None)
    spec_col = PartitionSpec(None, "core")
    specs_by_name = {}
    for nm in name_order:
        specs_by_name[nm] = spec_doc if nm.startswith("docm") else spec_col
    in_specs = tuple(specs_by_name[nm] for nm in name_order)
    sharded = jax.jit(
        shard_map(_body, mesh=mesh, in_specs=in_specs, out_specs=(spec_col,),
                  check_rep=False),
        keep_unused=True,
    )
    from jax.sharding import NamedSharding

    _CACHE["key"] = key
    _CACHE["fn"] = sharded
    _CACHE["shard_doc"] = NamedSharding(mesh, spec_doc)
    _CACHE["shard_col"] = NamedSharding(mesh, spec_col)
    return sharded


def _cast_slab(d2_u16, flat_sel, m_sel):
    """flat_sel [8, T] global token rows; m_sel [8, T] int32 validity.
    Returns byte planes [8, 3, T, NGR] u8 of N = sum(y_k 27^k) + 2^23
    (pads -> all digits 0, i.e. N = 2^23)."""
    l0, l1, l2, l3, l4 = _get_luts()
    g16 = d2_u16[flat_sel]                    # [8, T, 128] u16
    n = l0[g16[..., 0:NGR]]
    n += l1[g16[..., NGR:2 * NGR]]
    n += l2[g16[..., 2 * NGR:3 * NGR]]
    n += l3[g16[..., 3 * NGR:4 * NGR]]
    n[..., 0:D - 4 * NGR] += l4[g16[..., 4 * NGR:D]]
    n *= m_sel[..., None]                     # pads -> 0 (all digits 0)
    n += NBIAS
    # little-endian byte planes: strided u8 view copies, no arithmetic
    u8v = n.view(np.uint8).reshape(n.shape + (4,))
    out = np.empty((n.shape[0], 3) + n.shape[1:], np.uint8)
    out[:, 0] = u8v[..., 0]
    out[:, 1] = u8v[..., 1]
    out[:, 2] = u8v[..., 2]
    return out


def _prep_q(q: np.ndarray) -> np.ndarray:
    """L2-normalize over D, transpose to [D, B*LQ], cast to e4m3."""
    nrm = np.sqrt((q.astype(np.float64) ** 2).sum(-1, keepdims=True))
    qn = (q / np.maximum(nrm, 1e-12)).astype(np.float32)
    qt = np.ascontiguousarray(qn.transpose(2, 0, 1).reshape(D, B * LQ))
    return qt.astype(E4M3)


def _tail(scores: np.ndarray, lab: np.ndarray) -> np.float32:
    """softmax / KL / focal on [B, NWAY] in float64."""
    sc = scores.astype(np.float64)
    m = sc.max(-1, keepdims=True)
    ls = np.log(np.exp(sc - m).sum(-1, keepdims=True)) + m
    logp = sc - ls
    p = np.exp(logp)
    t = lab[:, :NWAY].astype(np.float64)
    kl = np.exp(t) * (t - logp)
    lv = kl * p ** GAMMA
    return np.float32(lv.mean())


def run(inputs, trace=False):
    q = np.asarray(inputs["query_reps"], dtype=np.float32)
    doc = np.ascontiguousarray(
        np.asarray(inputs["doc_reps"], dtype=np.float32)
    )
    msk = np.asarray(inputs["doc_masks"])
    lab = np.asarray(inputs["labels"], dtype=np.float32)

    if "base" not in _CACHE:
        _CACHE["base"] = (
            (np.arange(NWAY, dtype=np.int32)[:, None, None] * B
             + np.arange(B, dtype=np.int32)[None, :, None]) * LD
        )
    for attempt in range(3):
        loss = _attempt(q, doc, msk, lab)
        # the axon tunnel rarely corrupts a transfer (observed transient
        # NaN); the loss is one scalar, so detect and retry
        if np.isfinite(loss):
            break

    class _Res:
        results = None
        instructions_and_trace = None
        profile_json = None
        exec_time_ns = None

    return np.array(loss, dtype=np.float32), _Res()


def _attempt(q, doc, msk, lab):
    base = _CACHE["base"]
    # per-pair shared lengths: max unmasked count over the 8 cores
    cnt = msk.sum(-1, dtype=np.int32)                  # [NWAY, B]
    L = cnt.reshape(NWAY, NCORES, BSL).max(1)          # [NWAY, BSL]
    fn = _get_runner(L)
    if "zeros_dev" not in _CACHE:
        _CACHE["zeros_dev"] = jax.device_put(
            np.zeros((NWAY, B), np.float32), _CACHE["shard_col"])
    # top 16 bits of each f32 as strided rows [NWAY*B*LD, D]
    d2_u16 = doc.view(np.uint16)[..., 1::2].reshape(-1, D)
    # slab 0 preps UNCONTENDED and its put opens the wire ~17ms sooner;
    # q's small put then rides the stream (issued right after slab 0)
    qt_dev = None
    slabs = []
    ar = np.arange(LD, dtype=np.int32)
    for kk in range(NSLAB):
        sl = slice(kk * NL, (kk + 1) * NL)  # docs {2kk, 2kk+1}, all b
        Ls = L[sl]                                      # [NL, BSL]
        lmax = int(Ls.max())
        # regroup the mask to core-major [8, NL, BSL, LD] FIRST (small,
        # contiguous copy) so argsort/take outputs need no big transposes
        msk_t = np.ascontiguousarray(
            msk[sl].reshape(NL, NCORES, BSL, LD).transpose(1, 0, 2, 3))
        # stable sort puts ALL unmasked tokens first, in original order
        o = np.argsort(1 - msk_t, axis=-1, kind="stable")[..., :lmax]
        o = o.astype(np.int32)
        m16 = np.take_along_axis(msk_t, o, axis=-1).astype(np.int32)
        # global token-row base per (core, nl, bl)
        bt = ((kk * NL + np.arange(NL, dtype=np.int32))[None, :, None] * B
              + (np.arange(NCORES, dtype=np.int32)[:, None, None] * BSL
                 + np.arange(BSL, dtype=np.int32)[None, None, :])) * LD
        flat = bt[..., None] + o                        # [8, NL, BSL, lmax]
        # ragged-select the first L[nl, bl] positions of each pair
        sel = ar[None, None, :lmax] < Ls[:, :, None]    # [NL, BSL, lmax]
        flat_sel = flat[:, sel]                         # [8, T]
        m_sel = m16[:, sel]
        p16 = _cast_slab(d2_u16, flat_sel, m_sel)       # [8, 3, T, NGR]
        slabs.append(jax.device_put(p16, _CACHE["shard_doc"]))
        if qt_dev is None:
            qt_dev = jax.device_put(_prep_q(q), _CACHE["shard_col"])
    (outg,) = fn(*slabs, qt_dev, _CACHE["zeros_dev"])
    scores = np.asarray(outg).astype(np.float32).T  # [B, NWAY]
    return _tail(scores, lab)


def kernel(**inputs) -> np.ndarray:
    out, _ = run(inputs)
    return out
